# revision 3
# baseline (speedup 1.0000x reference)
"""GwcVolume (group-wise correlation cost volume) Trainium2 kernel.

cost[b,g,d,h,x] = mean_c( lf[b, g*8+c, h, x] * rf[b, g*8+c, h, x-d] ),
zero for x < d.  Shapes: lf/rf [2, 320, 128, 240] f32 -> out [2, 40, 48, 128, 240] f32.

Sharding: h-axis split across 8 cores (16 rows each). Correlation is along w
only, so shards are fully independent and each core reads just its h-band.

Per-core engine mapping:
  - SWDGE DMA loads lf/rf h-band, casting fp32->fp16 in flight. rf is stored
    with a 48-column zero left-pad (plus a second copy padded by 49+1 so that
    odd-d windows stay 4B-aligned and the DVE 2x perf mode always engages).
    The pad also makes the x<d output region exactly zero for free.
  - VectorE computes the elementwise products lf[c,x]*rf[c,x-d] in fp16 (2x).
  - TensorE reduces the 8 channels of each group via a constant 0/1*(1/8)
    block-diagonal stationary matrix; the three 16/16/8-group chunks are
    col-packed at tile_position (0,0)/(0,32)/(0,64) so PSUM output lands on
    72 partitions and can be drained in one instruction.
  - ScalarE drains PSUM->SBUF; HWDGE DMA writes h/x-contiguous output tiles.
"""

import numpy as np

import concourse.bass as bass
import concourse.tile as tile
from concourse import mybir
from concourse.bass_utils import run_bass_kernel_spmd

B = 2
C = 320
H = 128
W = 240
G = 40
CPG = 8
D = 48
NCORES = 8
HS = H // NCORES  # h rows per core
HB = 8  # h rows per inner block
F16 = mybir.dt.float16
F32 = mybir.dt.float32

# (channel offset, channel count, group count, psum col strip)
CHUNKS = [(0, 128, 16, 0), (128, 128, 16, 32), (256, 64, 8, 64)]


def split_multi_waits(nc, limit=1):
    """Walrus in this container rejects instructions carrying more than
    `limit` semaphore waits. Move excess waits onto preceding NoOps on the
    same engine (waits execute before the instruction, in stream order)."""
    n_split = 0
    for fn in nc.m.functions:
        for bb in fn.blocks:
            insts = bb.instructions
            i = 0
            while i < len(insts):
                inst = insts[i]
                si = inst.sync_info
                if si is not None and len(si.on_wait) > limit:
                    waits = list(si.on_wait)
                    keep = waits[-limit:]
                    extra = waits[:-limit]
                    new_insts = []
                    for j in range(0, len(extra), limit):
                        chunk = extra[j : j + limit]
                        nop = mybir.InstNoOp(
                            name=nc.get_next_instruction_name(),
                            engine=inst.engine,
                            ins=[],
                            outs=[],
                            sync_info=mybir.SyncInfo(on_wait=chunk, on_update=[]),
                        )
                        new_insts.append(nop)
                    inst.sync_info = mybir.SyncInfo(
                        on_wait=keep, on_update=list(si.on_update)
                    )
                    insts[i:i] = new_insts
                    i += len(new_insts)
                    n_split += 1
                i += 1
    return n_split


def build_bass(n_b=B, n_hb=HS // HB, n_d=D):
    nc = bass.Bass("TRN2", target_bir_lowering=False, debug=False, num_devices=NCORES)
    lf = nc.dram_tensor("lf", [B, C, HS, W], F32, kind="ExternalInput").ap()
    rf = nc.dram_tensor("rf", [B, C, HS, W], F32, kind="ExternalInput").ap()
    s16 = nc.dram_tensor("s16", [128, 16], F16, kind="ExternalInput").ap()
    s8 = nc.dram_tensor("s8", [64, 8], F16, kind="ExternalInput").ap()
    outp = nc.dram_tensor("outp", [B, G, D, HS, W], F32, kind="ExternalOutput").ap()

    with tile.TileContext(nc) as tc:
        with (
            tc.tile_pool(name="const", bufs=1) as cpool,
            tc.tile_pool(name="lf", bufs=2) as lpool,
            tc.tile_pool(name="rf", bufs=2) as rpool,
            tc.tile_pool(name="prod", bufs=3) as ppool,
            tc.tile_pool(name="outs", bufs=3) as opool,
            tc.tile_pool(name="psum", bufs=2, space="PSUM") as qpool,
        ):
            s16_t = cpool.tile([128, 16], F16)
            nc.sync.dma_start(s16_t[:], s16[:])
            s8_t = cpool.tile([64, 8], F16)
            nc.sync.dma_start(s8_t[:], s8[:])
            s_tiles = [s16_t, s16_t, s8_t]

            for b in range(n_b):
                for hg in range(n_hb):
                    h0 = hg * HB
                    lf_ts, rf_e_ts, rf_o_ts = [], [], []
                    for c0, cs, gsz, strip in CHUNKS:
                        lt = lpool.tile([cs, HB, W], F16, tag=f"lf{c0}")
                        nc.gpsimd.dma_start(
                            lt[:, :, :], lf[b, c0 : c0 + cs, h0 : h0 + HB, :]
                        )
                        lf_ts.append(lt)
                        re = rpool.tile([cs, HB, 48 + W], F16, tag=f"rfe{c0}")
                        nc.gpsimd.memset(re[:, :, 0:48], 0.0)
                        nc.gpsimd.dma_start(
                            re[:, :, 48 : 48 + W], rf[b, c0 : c0 + cs, h0 : h0 + HB, :]
                        )
                        rf_e_ts.append(re)
                        ro = rpool.tile([cs, HB, 50 + W], F16, tag=f"rfo{c0}")
                        nc.gpsimd.memset(ro[:, :, 0:49], 0.0)
                        nc.gpsimd.dma_start(
                            ro[:, :, 49 : 49 + W], rf[b, c0 : c0 + cs, h0 : h0 + HB, :]
                        )
                        rf_o_ts.append(ro)

                    for d in range(n_d):
                        prods = []
                        for ci, (c0, cs, gsz, strip) in enumerate(CHUNKS):
                            pt = ppool.tile([cs, HB, W], F16, tag=f"prod{c0}")
                            if d % 2 == 0:
                                rw = rf_e_ts[ci][:, :, 48 - d : 48 - d + W]
                            else:
                                rw = rf_o_ts[ci][:, :, 49 - d : 49 - d + W]
                            nc.vector.tensor_mul(pt[:, :, :], lf_ts[ci][:, :, :], rw)
                            prods.append(pt)
                        ps = qpool.tile([72, HB, 256], F32)
                        for ci, (c0, cs, gsz, strip) in enumerate(CHUNKS):
                            for j in range(HB // 2):
                                nc.tensor.matmul(
                                    ps[strip : strip + gsz, 2 * j : 2 * j + 2, 0:W],
                                    s_tiles[ci][:, :],
                                    prods[ci][:, 2 * j : 2 * j + 2, :],
                                    start=True,
                                    stop=True,
                                    tile_position=(0, strip),
                                )
                        ot = opool.tile([72, HB, W], F32)
                        nc.scalar.copy(ot[:, :, :], ps[:, :, 0:W])
                        for ci, (c0, cs, gsz, strip) in enumerate(CHUNKS):
                            g0 = 16 * ci
                            nc.sync.dma_start(
                                outp[b, g0 : g0 + gsz, d, h0 : h0 + HB, :],
                                ot[strip : strip + gsz, :, :],
                            )
    split_multi_waits(nc)
    return nc


def make_smats():
    s16 = np.zeros((128, 16), np.float16)
    for g in range(16):
        s16[g * CPG : (g + 1) * CPG, g] = 1.0 / CPG
    s8 = np.zeros((64, 8), np.float16)
    for g in range(8):
        s8[g * CPG : (g + 1) * CPG, g] = 1.0 / CPG
    return s16, s8


_NC_CACHE = {}


def _get_nc(key=(B, HS // HB, D)):
    if key not in _NC_CACHE:
        _NC_CACHE[key] = build_bass(*key)
    return _NC_CACHE[key]


def run_sharded(lf, rf, nc=None, trace=False, tmpdir=None):
    """lf/rf: full [2, 320, 128, 240] f32 numpy arrays. Returns (out, results)."""
    if nc is None:
        nc = _get_nc()
    s16, s8 = make_smats()
    in_maps = []
    for k in range(NCORES):
        in_maps.append(
            {
                "lf": np.ascontiguousarray(lf[:, :, k * HS : (k + 1) * HS, :]),
                "rf": np.ascontiguousarray(rf[:, :, k * HS : (k + 1) * HS, :]),
                "s16": s16,
                "s8": s8,
            }
        )
    res = run_bass_kernel_spmd(
        nc, in_maps, list(range(NCORES)), trace=trace, tmpdir=tmpdir
    )
    out = np.empty((B, G, D, H, W), np.float32)
    for k in range(NCORES):
        out[:, :, :, k * HS : (k + 1) * HS, :] = res.results[k]["outp"]
    return out, res


def kernel(**inputs):
    lf = np.asarray(inputs["left_feature"], dtype=np.float32)
    rf = np.asarray(inputs["right_feature"], dtype=np.float32)
    out, _ = run_sharded(lf, rf)
    return out


if __name__ == "__main__":
    rng = np.random.default_rng(0)
    lf = rng.standard_normal((B, C, H, W), dtype=np.float32)
    rf = rng.standard_normal((B, C, H, W), dtype=np.float32)
    out, _ = run_sharded(lf, rf)
    print(out.shape, out.dtype, float(np.abs(out).max()))

